# revision 12
# baseline (speedup 1.0000x reference)
"""Trainium2 Bass kernel for nn_CAN_Layer_74775380623980.

Math: with sequence length L=1, softmax over the single key is exactly 1.0
and the reference's masks are overwritten with ones, so the whole cross
attention collapses to

    E   = (protein @ Wv_p + drug @ Wv_d) / 2          # [N, 2048]
    out = concat([E, E], axis=1)                      # [N, 4096]

Sharding: pure data parallel, batch N=16384 split 8 ways (2048 rows/core);
the two V projection weights are replicated.

Precision/speed: per (m-tile, n-block) PSUM accumulation runs the first
1536 of the K=2048 contraction as fp16 matmuls (12 k-strips of 128) and the
last 512 as two fp8-e4m3 DoubleRow matmuls (2 k-strips of 256, 2x PE
throughput). fp8 operands are pre-scaled on host by exact powers of two
(w*32, x/32) so the product scale matches the fp16 strips sharing the same
PSUM bank. Measured rel err vs the fp32 reference: ~1.9e-2 (deterministic
inputs), under the 2e-2 gate; pure-fp16 would be 3e-4 but ~11% slower.
"""

import numpy as np

P = 128          # partitions / systolic tile
N_FULL = 16384
D = 2048         # contraction dim
HID = 2048       # output dim per projection
NCORES = 8
M_SH = N_FULL // NCORES   # 2048 rows per core
KD = 2                    # fp8 DoubleRow k-strips (256 each)
KT16 = 16 - 2 * KD        # fp16 k-strips (128 each)
K16 = KT16 * P
XSC = 16.0                # fp8 scale split: w*XSC, x/XSC (exact pow2)
NBLK = 512                # matmul free dim (one PSUM bank of fp32)
NB = HID // NBLK          # 4 n-blocks
MT_FULL = M_SH // P       # 16 m-tiles


def _build_module(mt_tiles=MT_FULL, reps=1, xbufs=2, obufs=2, paired=False):
    """reps>1 wraps the body in a device-side For_i — used only for
    wall-clock benchmarking (amplifies device time above RPC noise)."""
    import concourse.bass as bass  # noqa: F401
    import concourse.mybir as mybir
    import concourse.tile as tile
    from concourse import bacc

    fp16 = mybir.dt.float16
    fp8 = mybir.dt.float8e4
    f32 = mybir.dt.float32
    DR = mybir.MatmulPerfMode.DoubleRow

    nc = bacc.Bacc("TRN2", target_bir_lowering=False, debug=False)

    xp_h = nc.dram_tensor("xp", [mt_tiles, P, KT16, P], fp16, kind="ExternalInput")
    xd_h = nc.dram_tensor("xd", [mt_tiles, P, KT16, P], fp16, kind="ExternalInput")
    xp8_h = nc.dram_tensor("xp8", [mt_tiles, P, KD, 2, P], fp8, kind="ExternalInput")
    xd8_h = nc.dram_tensor("xd8", [mt_tiles, P, KD, 2, P], fp8, kind="ExternalInput")
    wp_h = nc.dram_tensor("wp", [KT16, P, HID], fp16, kind="ExternalInput")
    wd_h = nc.dram_tensor("wd", [KT16, P, HID], fp16, kind="ExternalInput")
    wp8_h = nc.dram_tensor("wp8", [KD, P, 2, HID], fp8, kind="ExternalInput")
    wd8_h = nc.dram_tensor("wd8", [KD, P, 2, HID], fp8, kind="ExternalInput")
    out_h = nc.dram_tensor("out", [mt_tiles * P, HID], f32, kind="ExternalOutput")

    with tile.TileContext(nc) as tc:
        with (
            tc.tile_pool(name="wpool", bufs=1) as wpool,
            tc.tile_pool(name="xpool", bufs=2 * xbufs) as xpool,
            tc.tile_pool(name="opool", bufs=obufs) as opool,
            tc.tile_pool(name="psum", bufs=(1 if paired else 2), space="PSUM") as pp,
        ):
            x_tiles = {}

            def load_x(mt):
                tp = xpool.tile([P, KT16, P], fp16, tag="xp", name=f"xp_{mt}")
                nc.sync.dma_start(tp[:], xp_h[mt])
                tp8 = xpool.tile([P, KD, 2, P], fp8, tag="xp8", name=f"xp8_{mt}")
                nc.sync.dma_start(tp8[:], xp8_h[mt])
                td = xpool.tile([P, KT16, P], fp16, tag="xd", name=f"xd_{mt}")
                nc.sync.dma_start(td[:], xd_h[mt])
                td8 = xpool.tile([P, KD, 2, P], fp8, tag="xd8", name=f"xd8_{mt}")
                nc.sync.dma_start(td8[:], xd8_h[mt])
                x_tiles[mt] = (tp, td, tp8, td8)

            wp_sb, wd_sb, wp8_sb, wd8_sb = [], [], [], []

            def load_weights():
                # weights stream on the Act HWDGE ring (nc.scalar) so the
                # 14MB burst doesn't block x/out traffic on the SP ring —
                # HWDGE DMAs are FIFO per issuing engine
                wp_sb.clear()
                wd_sb.clear()
                wp8_sb.clear()
                wd8_sb.clear()
                for j in range(KT16):
                    tw = wpool.tile([P, HID], fp16, tag=f"wp{j}", name=f"wp_{j}")
                    nc.scalar.dma_start(tw[:], wp_h[j])
                    wp_sb.append(tw)
                    tw = wpool.tile([P, HID], fp16, tag=f"wd{j}", name=f"wd_{j}")
                    nc.scalar.dma_start(tw[:], wd_h[j])
                    wd_sb.append(tw)
                for j in range(KD):
                    tw = wpool.tile([P, 2, HID], fp8, tag=f"wp8{j}", name=f"wp8_{j}")
                    nc.scalar.dma_start(tw[:], wp8_h[j])
                    wp8_sb.append(tw)
                    tw = wpool.tile([P, 2, HID], fp8, tag=f"wd8{j}", name=f"wd8_{j}")
                    nc.scalar.dma_start(tw[:], wd8_h[j])
                    wd8_sb.append(tw)

            next_load = [1]  # load_x(0) is issued before the loop

            def ensure_loads(upto):
                while next_load[0] <= min(upto, mt_tiles - 1):
                    load_x(next_load[0])
                    next_load[0] += 1

            def m_loop_paired():
                # two m-tiles in flight per j-step (8 PSUM banks): during the
                # cold-start weight stream-in, the PE consumes each arriving
                # k-strip with 2x the matmul work, eliminating starvation
                assert mt_tiles % 2 == 0, "paired m-loop needs an even m-tile count"
                next_load[0] = 1
                for mt0 in range(0, mt_tiles, 2):
                    pair = [mt0, mt0 + 1]
                    ensure_loads(mt0 + 1 + 2 * (xbufs - 1))
                    xt = {mt: x_tiles.pop(mt) for mt in pair}
                    psums = {
                        (h, nb): pp.tile(
                            [P, NBLK], f32, tag=f"ps{h}_{nb}", name=f"ps_{mt0}_{h}_{nb}"
                        )
                        for h in range(2)
                        for nb in range(NB)
                    }
                    for j in range(KT16):
                        for h, mt in enumerate(pair):
                            for nb in range(NB):
                                nc.tensor.matmul(
                                    psums[h, nb][:],
                                    xt[mt][0][:, j, :],
                                    wp_sb[j][:, nb * NBLK : (nb + 1) * NBLK],
                                    start=(j == 0),
                                    stop=False,
                                )
                        for h, mt in enumerate(pair):
                            for nb in range(NB):
                                nc.tensor.matmul(
                                    psums[h, nb][:],
                                    xt[mt][1][:, j, :],
                                    wd_sb[j][:, nb * NBLK : (nb + 1) * NBLK],
                                    start=False,
                                    stop=False,
                                )
                    # fp8 DoubleRow tail: KD strips of K=256 per projection
                    for jd in range(KD):
                        for h, mt in enumerate(pair):
                            for nb in range(NB):
                                nc.tensor.matmul(
                                    psums[h, nb][:],
                                    xt[mt][2][:, jd],
                                    wp8_sb[jd][:, :, nb * NBLK : (nb + 1) * NBLK],
                                    start=(KT16 == 0 and jd == 0),
                                    stop=False,
                                    perf_mode=DR,
                                    skip_group_check=True,
                                )
                        for h, mt in enumerate(pair):
                            for nb in range(NB):
                                nc.tensor.matmul(
                                    psums[h, nb][:],
                                    xt[mt][3][:, jd],
                                    wd8_sb[jd][:, :, nb * NBLK : (nb + 1) * NBLK],
                                    start=False,
                                    stop=(jd == KD - 1),
                                    perf_mode=DR,
                                    skip_group_check=True,
                                )
                    for h, mt in enumerate(pair):
                        out_t = opool.tile([P, HID], f32, tag="out", name=f"out_{mt}")
                        for nb in range(NB):
                            nc.vector.tensor_copy(
                                out_t[:, nb * NBLK : (nb + 1) * NBLK], psums[h, nb][:]
                            )
                        nc.sync.dma_start(out_h[mt * P : (mt + 1) * P, :], out_t[:])

            def m_loop_single():
                # one m-tile per group: 4 PSUM banks per set, pool bufs=2
                # double-buffers the sets so the DVE drain of tile i overlaps
                # the matmuls of tile i+1 (paired mode uses all 8 banks and
                # stalls ~2us per pair boundary waiting on the DVE)
                next_load[0] = 1
                for mt in range(mt_tiles):
                    ensure_loads(mt + xbufs - 1)
                    xt = x_tiles.pop(mt)
                    psums = [
                        pp.tile([P, NBLK], f32, tag=f"ps{nb}", name=f"ps_{mt}_{nb}")
                        for nb in range(NB)
                    ]
                    for j in range(KT16):
                        for nb in range(NB):
                            nc.tensor.matmul(
                                psums[nb][:],
                                xt[0][:, j, :],
                                wp_sb[j][:, nb * NBLK : (nb + 1) * NBLK],
                                start=(j == 0),
                                stop=False,
                            )
                        for nb in range(NB):
                            nc.tensor.matmul(
                                psums[nb][:],
                                xt[1][:, j, :],
                                wd_sb[j][:, nb * NBLK : (nb + 1) * NBLK],
                                start=False,
                                stop=False,
                            )
                    for jd in range(KD):
                        for nb in range(NB):
                            nc.tensor.matmul(
                                psums[nb][:],
                                xt[2][:, jd],
                                wp8_sb[jd][:, :, nb * NBLK : (nb + 1) * NBLK],
                                start=(KT16 == 0 and jd == 0),
                                stop=False,
                                perf_mode=DR,
                                skip_group_check=True,
                            )
                        for nb in range(NB):
                            nc.tensor.matmul(
                                psums[nb][:],
                                xt[3][:, jd],
                                wd8_sb[jd][:, :, nb * NBLK : (nb + 1) * NBLK],
                                start=False,
                                stop=(jd == KD - 1),
                                perf_mode=DR,
                                skip_group_check=True,
                            )
                    out_t = opool.tile([P, HID], f32, tag="out", name=f"out_{mt}")
                    for nb in range(NB):
                        nc.vector.tensor_copy(
                            out_t[:, nb * NBLK : (nb + 1) * NBLK], psums[nb][:]
                        )
                    nc.sync.dma_start(out_h[mt * P : (mt + 1) * P, :], out_t[:])

            body = m_loop_paired if paired else m_loop_single
            if reps == 1:
                # first m-tile's activations go out before the weight preload
                # so the PE starts as soon as strip j=0 of the weights lands
                load_x(0)
                load_weights()
                body()
            else:
                # full body (weight preload included) repeats: per-rep wall
                # time == one-shot kernel exec time
                with tc.For_i(0, reps, 1):
                    load_x(0)
                    load_weights()
                    body()

    nc.compile()
    return nc


USE_GPTQ = KD > 2  # error-feedback quantization buys the bigger fp8 share


def _gptq_rows(V, Hm, q8, lam=0.01):
    """Quantize rows of V [R, T] to fp8 with GPTQ error feedback, minimizing
    ||(V - Q) @ B||_F where Hm = B @ B.T [T, T]."""
    T = Hm.shape[0]
    Hd = Hm.astype(np.float64) + np.eye(T) * (lam * float(np.mean(np.diag(Hm))))
    Hinv = np.linalg.inv(Hd)
    U = np.linalg.cholesky(Hinv).T.astype(np.float32)  # upper; Hinv = U.T @ U
    V = V.astype(np.float32).copy()
    Q = np.empty_like(V)
    for i in range(T):
        qi = q8(V[:, i])
        Q[:, i] = qi
        err = (V[:, i] - qi) * (1.0 / U[i, i])
        if i + 1 < T:
            V[:, i + 1 :] -= err[:, None] * U[i, i + 1 :][None, :]
    return Q


def _prep_inputs(protein, drug, Wv_p, Wv_d, mt_tiles=MT_FULL):
    """Host-side shard + transpose-tile + fp16/fp8 cast."""
    import ml_dtypes

    e4m3 = ml_dtypes.float8_e4m3

    wp = 0.5 * np.asarray(Wv_p, dtype=np.float32)
    wd = 0.5 * np.asarray(Wv_d, dtype=np.float32)

    protein = np.asarray(protein, dtype=np.float32)
    drug = np.asarray(drug, dtype=np.float32)

    q8f = lambda a: np.asarray(a, np.float32).astype(e4m3).astype(np.float32)

    def quant_tail(w, x):
        """fp8 tail quantization for one projection: returns (Wq [T,HID],
        Xq [N,T]) as fp32 arrays holding exactly-representable e4m3 values."""
        Vw = w[K16:] * XSC
        Vx = x[:, K16:] * (1.0 / XSC)
        if not USE_GPTQ:
            return q8f(Vw), q8f(Vx)
        Wq = q8f(Vw)
        Xq = _gptq_rows(Vx, Wq @ Wq.T, q8f)
        Wq = _gptq_rows(np.ascontiguousarray(Vw.T), Xq.T @ Xq, q8f).T
        return np.ascontiguousarray(Wq), Xq

    wq_p, xq_p = quant_tail(wp, protein)
    wq_d, xq_d = quant_tail(wd, drug)

    def prep_w(w, wq):
        w16 = np.ascontiguousarray(
            w[:K16].astype(np.float16).reshape(KT16, P, HID)
        )
        # [KD, P, 2, HID]: w8[jd, p, i, n] = wq[jd*256 + i*128 + p, n]
        w8 = wq.astype(e4m3).reshape(KD, 2, P, HID).transpose(0, 2, 1, 3)
        return w16, np.ascontiguousarray(w8)

    wp16, wp8 = prep_w(wp, wq_p)
    wd16, wd8 = prep_w(wd, wq_d)

    def tile_x16(x):
        # [rows, K16] -> [mt, p, j, m'] with t[mt, p, j, m'] = x[mt*P+m', j*P+p]
        t = x[:, :K16].reshape(mt_tiles, P, KT16, P).transpose(0, 3, 2, 1)
        return np.ascontiguousarray(t.astype(np.float16))

    def tile_x8(xq):
        # [rows, T] quantized tail -> [mt, p, jd, i, m']:
        # t[mt, p, jd, i, m'] = xq[mt*P+m', jd*256 + i*128 + p]
        t = xq.astype(e4m3)
        t = t.reshape(mt_tiles, P, KD, 2, P).transpose(0, 4, 2, 3, 1)
        return np.ascontiguousarray(t)
    in_maps = []
    rows = mt_tiles * P
    for c in range(NCORES):
        sl = slice(c * M_SH, c * M_SH + rows)
        in_maps.append(
            {
                "xp": tile_x16(protein[sl]),
                "xd": tile_x16(drug[sl]),
                "xp8": tile_x8(xq_p[sl]),
                "xd8": tile_x8(xq_d[sl]),
                "wp": wp16,
                "wd": wd16,
                "wp8": wp8,
                "wd8": wd8,
            }
        )
    return in_maps


_MODULE_CACHE = {}


def _run(protein, drug, Wv_p, Wv_d, trace=False, mt_tiles=MT_FULL):
    from concourse.bass_utils import run_bass_kernel_spmd

    nc = _MODULE_CACHE.get(mt_tiles)
    if nc is None:
        nc = _MODULE_CACHE[mt_tiles] = _build_module(mt_tiles)
    in_maps = _prep_inputs(protein, drug, Wv_p, Wv_d, mt_tiles)
    res = run_bass_kernel_spmd(nc, in_maps, list(range(NCORES)), trace=trace)
    E = np.concatenate(
        [np.asarray(r["out"], dtype=np.float32) for r in res.results], axis=0
    )
    return E, res


def kernel(
    protein,
    drug,
    mask_prot=None,
    mask_drug=None,
    Wq_p=None,
    Wk_p=None,
    Wv_p=None,
    Wq_d=None,
    Wk_d=None,
    Wv_d=None,
):
    E, _ = _run(protein, drug, Wv_p, Wv_d, trace=False)
    return np.concatenate([E, E], axis=1)


def kernel_profiled(**inputs):
    E, res = _run(
        inputs["protein"], inputs["drug"], inputs["Wv_p"], inputs["Wv_d"], trace=False
    )
    out = np.concatenate([E, E], axis=1)
    return out, res
